# revision 43
# baseline (speedup 1.0000x reference)
"""Trainium2 Bass kernel for nn_AttentionModule (B=1024, F=512, N=8).

Pure data-parallel across 8 NeuronCores: batch is sharded 8 x 128, all
weights replicated. No collectives.

Algebraic restructuring (exact, verified vs the jax reference):
  - conv1's input t[b,f,i,j] = va[b,f,i] is constant along j, so the 3x3
    SAME conv over the 8x8 image has only 3 distinct column types
    (q=0 / interior / q=7).  Each type is a 3-tap 1-D conv over the
    height axis with dw-folded weights -> 9 [B*8,512]@[512,512] matmuls.
  - The 4x4/4 maxpool mixes column types pairwise and reduces height
    windows {0..3},{4..7}.
  - conv2 (3x3 SAME on 2x2) + mean + bias collapses to 4 matmuls with
    host-folded coefficient matrices C_pq plus a folded bias vector.
All matmul operands are bf16 (TensorE 1 cyc/row); accumulation f32.
"""

import numpy as np
import ml_dtypes

B, F, N = 1024, 512, 8
NCORES = 8
BL = B // NCORES  # 128 batch rows per core

_BF16 = ml_dtypes.bfloat16


def _bf16(a):
    return np.ascontiguousarray(np.asarray(a, np.float32).astype(_BF16))


def _chunk_pf(mat, free):
    """[K, free] (contraction-major) -> SBUF tile layout [128, (K//128)*free]
    where tile[kp, kc*free + f] = mat[kc*128 + kp, f]."""
    K = mat.shape[0]
    kc = K // 128
    return np.ascontiguousarray(
        mat.reshape(kc, 128, free).transpose(1, 0, 2).reshape(128, kc * free)
    )


def _apply_tile_drain_patch():
    """The neuronxcc walrus in this container rejects instructions with >1
    sync wait (CoreV3 setupSyncWait 'Too many sync wait commands').  Split
    the Tile exit drain's waits across multiple drain instructions."""
    import bass_rust
    import concourse.tile as tile
    from concourse.vector_clock import ScopedClock

    MAXW = 1

    def _drain_and_barrier_split(self, tick_clock, wait_clock):
        nc = self.nc
        drain_inst = nc.sync.drain()
        wait_clock.add_sem_waits(
            drain_inst.ins, ScopedClock({None: tick_clock.global_clock})
        )
        si = drain_inst.ins.sync_info
        waits = list(si.on_wait) if si is not None else []
        if len(waits) > MAXW:
            si.on_wait = waits[:MAXW]
            drain_inst.ins.sync_info = si
            for i in range(MAXW, len(waits), MAXW):
                extra = nc.sync.drain()
                esi = extra.ins.sync_info
                if esi is None:
                    esi = bass_rust.SyncInfo(on_wait=[], on_update=[])
                esi.on_wait = waits[i : i + MAXW]
                extra.ins.sync_info = esi
        nc.all_engine_barrier()
        assert self.sems is not None
        popped = nc._tile_sem_poison_stack.pop()
        assert popped is self._sem_poison
        nc.clear_and_free_semaphores(list(self.sems.allocated().values()))
        nc.all_engine_barrier()

    tile.TileContext._drain_and_barrier = _drain_and_barrier_split


def _split_multi_waits(nc, maxw=1):
    """neuronxcc walrus accepts at most one sync wait per instruction.
    For any instruction carrying more, keep the first wait and hoist the
    rest onto nop instructions inserted immediately before it on the same
    engine."""
    import bass_rust

    for f in nc.m.functions:
        for bb in f.blocks:
            insts = bb.instructions  # live list
            multi = []
            for idx, ins in enumerate(insts):
                si = ins.sync_info
                if si is not None and len(si.on_wait) > maxw:
                    multi.append(idx)
            if not multi:
                continue
            new_list = []
            for idx, ins in enumerate(insts):
                si = ins.sync_info
                waits = list(si.on_wait) if si is not None else []
                if len(waits) > maxw:
                    for j in range(maxw, len(waits), maxw):
                        beng = nc.engines[ins.engine]
                        bi = beng.nop(nofuse=True)
                        cur = nc.cur_bb.bb
                        assert cur.instructions[-1].name == bi.ins.name
                        cur.instructions.pop()
                        nsi = bi.ins.sync_info
                        if nsi is None:
                            nsi = bass_rust.SyncInfo(on_wait=[], on_update=[])
                        nsi.on_wait = waits[j : j + maxw]
                        bi.ins.sync_info = nsi
                        new_list.append(bi.ins)
                    si.on_wait = waits[:maxw]
                    ins.sync_info = si
                new_list.append(ins)
            insts[:] = new_list


def _build_program(with_b1, with_b2, with_bf):
    import concourse.bass as bass
    import concourse.mybir as mybir
    import concourse.tile as tile
    from concourse.masks import make_identity

    _apply_tile_drain_patch()

    bf = mybir.dt.bfloat16
    f32 = mybir.dt.float32
    AF = mybir.ActivationFunctionType
    ALU = mybir.AluOpType

    nc = bass.Bass()
    xt_e = nc.declare_dram_parameter("xt", [128, 8 * 128], bf, isOutput=False)
    vx_e = nc.declare_dram_parameter("vx", [128, 512], bf, isOutput=False)
    w1_e = nc.declare_dram_parameter("w1t", [8, 128, 8 * 512], bf, isOutput=False)
    w2_e = nc.declare_dram_parameter("w2t", [8, 128, 4 * 512], bf, isOutput=False)
    wc_e = nc.declare_dram_parameter("wc", [9, 128, 4 * 512], bf, isOutput=False)
    c4_e = nc.declare_dram_parameter("c4", [4, 128, 4 * 512], bf, isOutput=False)
    if with_b1:
        b1_e = nc.declare_dram_parameter("b1r", [128, 8 * 512], f32, isOutput=False)
    if with_b2:
        b2_e = nc.declare_dram_parameter("b2r", [128, 8 * 512], f32, isOutput=False)
    if with_bf:
        bf_e = nc.declare_dram_parameter("bfr", [128, 512], f32, isOutput=False)
    out_e = nc.declare_dram_parameter("out", [128, 512], f32, isOutput=True)

    with tile.TileContext(nc) as tc:
        with (
            tc.tile_pool(name="const", bufs=1) as constp,
            tc.tile_pool(name="wgt1", bufs=3) as w1p,
            tc.tile_pool(name="wgt2", bufs=3) as w2p,
            tc.tile_pool(name="wc", bufs=9) as wcp,
            tc.tile_pool(name="c4", bufs=4) as c4p,
            tc.tile_pool(name="act", bufs=2) as actp,
            tc.tile_pool(name="gt", bufs=4) as gtp,
            tc.tile_pool(name="pool", bufs=4) as poolp,
            tc.tile_pool(name="psA", bufs=2, space="PSUM") as psA,
            tc.tile_pool(name="psT", bufs=2, space="PSUM") as psT,
            tc.tile_pool(name="psY", bufs=4, space="PSUM") as psY,
        ):
            ident = constp.tile([128, 128], bf, tag="ident")
            make_identity(nc, ident[:])

            xt_sb = constp.tile([128, 8 * 128], bf, tag="xt")
            nc.sync.dma_start(xt_sb[:], xt_e[:])
            vx_sb = constp.tile([128, 512], bf, tag="vx")
            if with_b1:
                b1_sb = constp.tile([128, 8 * 512], f32, tag="b1")
                nc.sync.dma_start(b1_sb[:], b1_e[:])
            if with_b2:
                b2_sb = constp.tile([128, 8 * 512], f32, tag="b2")
                nc.sync.dma_start(b2_sb[:], b2_e[:])

            wc_sb = [None] * 9
            c4_sb = [None] * 4
            # wc DMA emission staggered between unit-weight DMAs so the
            # first phase-3 blocks (interleaved at p = n-2) have their
            # matrices without starving the unit-weight stream.
            wc_sched = {0: (1, 4, 7), 1: (2, 5, 8), 2: (0, 3, 6)}

            # GT_sb[cc][cp, n*128 + b] = G[b, n, cc*128+cp]  (bf16)
            gt_sb = [
                gtp.tile([128, 8 * 128], bf, tag="gt", name=f"gt{i}")
                for i in range(4)
            ]

            # incremental pooled maxes P00, P01, P10, P11  [128, 512] bf16
            P_sb = [
                poolp.tile([128, 512], bf, tag="P", name=f"P{i}") for i in range(4)
            ]

            def phase3_block(p):
                # y_T[p][b, o] = sum_{dh valid, cc} GT[cc][:, h]^T @ WC[T,dh][cc]
                dhs = [dh for dh in range(3) if 0 <= p - 1 + dh <= 7]
                ys = []
                for t in range(3):  # 0=L, 1=I, 2=R
                    py = psY.tile([128, 512], f32, tag="y", name=f"y{t}_{p}")
                    nmm = len(dhs) * 4
                    i = 0
                    for dh in dhs:
                        h = p - 1 + dh
                        for cc in range(4):
                            nc.tensor.matmul(
                                py[:],
                                gt_sb[cc][:, h * 128 : (h + 1) * 128],
                                wc_sb[t * 3 + dh][:, cc * 512 : (cc + 1) * 512],
                                start=(i == 0),
                                stop=(i == nmm - 1),
                            )
                            i += 1
                    ys.append(py)
                yL, yI, yR = ys
                # yI to SBUF so DVE maxes read one PSUM + one SBUF operand
                yI_sb = actp.tile([128, 512], f32, tag="yis", name=f"yis{p}")
                nc.scalar.activation(yI_sb[:], yI[:], AF.Copy)
                m0 = actp.tile([128, 512], bf, tag="m0", name=f"m0_{p}")
                nc.vector.tensor_tensor(m0[:], yL[:], yI_sb[:], op=ALU.max)
                m1 = actp.tile([128, 512], bf, tag="m1", name=f"m1_{p}")
                nc.vector.tensor_tensor(m1[:], yR[:], yI_sb[:], op=ALU.max)
                w = p // 4
                for idx, m in ((w * 2, m0), (w * 2 + 1, m1)):
                    if p % 4 == 0:
                        nc.vector.tensor_copy(P_sb[idx][:], m[:])
                    else:
                        nc.vector.tensor_tensor(
                            P_sb[idx][:], P_sb[idx][:], m[:], op=ALU.max
                        )

            # ---- phases 1+2 per attention unit n, phase 3 interleaved ----
            for n in range(8):
                w1_sb = w1p.tile([128, 8 * 512], bf, tag="w1")
                nc.sync.dma_start(w1_sb[:, :2048], w1_e[n][:, :2048])
                nc.sync.dma_start(w1_sb[:, 2048:], w1_e[n][:, 2048:])
                if n == 0:
                    nc.sync.dma_start(vx_sb[:], vx_e[:])
                w2_sb = w2p.tile([128, 4 * 512], bf, tag="w2")
                nc.sync.dma_start(w2_sb[:], w2_e[n])
                for i in wc_sched.get(n, ()):
                    t = wcp.tile([128, 4 * 512], bf, tag="wc", name=f"wct{i}")
                    nc.sync.dma_start(t[:], wc_e[i])
                    wc_sb[i] = t
                if n == 7:
                    for i in range(4):
                        t = c4p.tile([128, 4 * 512], bf, tag="c4", name=f"c4t{i}")
                        nc.sync.dma_start(t[:], c4_e[i])
                        c4_sb[i] = t
                    if with_bf:
                        bf_sb = constp.tile([128, 512], f32, tag="bf")
                        nc.sync.dma_start(bf_sb[:], bf_e[:])

                # phase 1: h[b, f] = relu(x @ W1[n].T + b1[n])  [128, 512]
                ph = psA.tile([128, 512], f32, tag="acc")
                for kc in range(8):
                    nc.tensor.matmul(
                        ph[:],
                        xt_sb[:, kc * 128 : (kc + 1) * 128],
                        w1_sb[:, kc * 512 : (kc + 1) * 512],
                        start=(kc == 0),
                        stop=(kc == 7),
                    )
                h_sb = actp.tile([128, 512], bf, tag="h")
                if with_b1:
                    htmp = actp.tile([128, 512], f32, tag="htmp")
                    nc.vector.tensor_add(htmp[:], ph[:], b1_sb[:, n * 512 : (n + 1) * 512])
                    nc.scalar.activation(h_sb[:], htmp[:], AF.Relu)
                else:
                    nc.scalar.activation(h_sb[:], ph[:], AF.Relu)

                # transpose h -> hT_sb[fp, fc*128 + b]
                ht_sb = actp.tile([128, 512], bf, tag="ht")
                for fc in range(4):
                    tp = psT.tile([128, 128], bf, tag="tp")
                    nc.tensor.transpose(
                        tp[:], h_sb[:, fc * 128 : (fc + 1) * 128], ident[:]
                    )
                    nc.vector.tensor_copy(ht_sb[:, fc * 128 : (fc + 1) * 128], tp[:])

                # phase 2: att[b, g] = relu(h @ W2[n].T + b2[n])
                pa = psA.tile([128, 512], f32, tag="acc")
                for fc in range(4):
                    nc.tensor.matmul(
                        pa[:],
                        ht_sb[:, fc * 128 : (fc + 1) * 128],
                        w2_sb[:, fc * 512 : (fc + 1) * 512],
                        start=(fc == 0),
                        stop=(fc == 3),
                    )
                att_sb = actp.tile([128, 512], bf, tag="att")
                if with_b2:
                    atmp = actp.tile([128, 512], f32, tag="atmp")
                    nc.vector.tensor_add(atmp[:], pa[:], b2_sb[:, n * 512 : (n + 1) * 512])
                    nc.scalar.activation(att_sb[:], atmp[:], AF.Relu)
                else:
                    nc.scalar.activation(att_sb[:], pa[:], AF.Relu)

                # G[b, c] = att[b, c] * vx[b, c]
                g_sb = actp.tile([128, 512], bf, tag="g")
                nc.vector.tensor_mul(g_sb[:], att_sb[:], vx_sb[:])

                # transpose G into GT_sb[cc][:, n*128:(n+1)*128]
                for cc in range(4):
                    tp = psT.tile([128, 128], bf, tag="tp")
                    nc.tensor.transpose(
                        tp[:], g_sb[:, cc * 128 : (cc + 1) * 128], ident[:]
                    )
                    nc.vector.tensor_copy(
                        gt_sb[cc][:, n * 128 : (n + 1) * 128], tp[:]
                    )

                # phase-3 block p needs GT columns up to h=p+1, i.e. units
                # 0..p+1 done -> run block n-2 here
                if n >= 2:
                    phase3_block(n - 2)

            phase3_block(6)
            phase3_block(7)

            # ---- phase 4: out = relu(sum_pq P_pq @ C_pq^T + bias) ----
            pt_sb = [
                poolp.tile([128, 512], bf, tag="PT", name=f"PT{i}") for i in range(4)
            ]
            for pq in range(4):
                for cc in range(4):
                    tp = psT.tile([128, 128], bf, tag="tp")
                    nc.tensor.transpose(
                        tp[:], P_sb[pq][:, cc * 128 : (cc + 1) * 128], ident[:]
                    )
                    nc.vector.tensor_copy(
                        pt_sb[pq][:, cc * 128 : (cc + 1) * 128], tp[:]
                    )
            po = psA.tile([128, 512], f32, tag="acc")
            i = 0
            for pq in range(4):
                for cc in range(4):
                    nc.tensor.matmul(
                        po[:],
                        pt_sb[pq][:, cc * 128 : (cc + 1) * 128],
                        c4_sb[pq][:, cc * 512 : (cc + 1) * 512],
                        start=(i == 0),
                        stop=(i == 15),
                    )
                    i += 1
            out_sb = constp.tile([128, 512], f32, tag="out")
            if with_bf:
                otmp = constp.tile([128, 512], f32, tag="otmp")
                nc.vector.tensor_add(otmp[:], po[:], bf_sb[:])
                nc.scalar.activation(out_sb[:], otmp[:], AF.Relu)
            else:
                nc.scalar.activation(out_sb[:], po[:], AF.Relu)
            nc.sync.dma_start(out_e[:], out_sb[:])

    _split_multi_waits(nc)
    return nc


def _prep_inputs(vx, ax, W1, b1, W2, b2, conv1_w, conv1_b, conv2_w, conv2_b):
    vx = np.asarray(vx, np.float32)
    ax = np.asarray(ax, np.float32)
    W1 = np.asarray(W1, np.float32)
    b1 = np.asarray(b1, np.float32)
    W2 = np.asarray(W2, np.float32)
    b2 = np.asarray(b2, np.float32)
    c1w = np.asarray(conv1_w, np.float32)
    c1b = np.asarray(conv1_b, np.float32)
    c2w = np.asarray(conv2_w, np.float32)
    c2b = np.asarray(conv2_b, np.float32)

    x = np.concatenate([vx, ax], axis=1)  # [B, 2F]

    # shared weights
    w1t = _bf16(
        W1.transpose(0, 2, 1).reshape(8, 8, 128, 512).transpose(0, 2, 1, 3)
        .reshape(8, 128, 8 * 512)
    )
    w2t = _bf16(
        W2.transpose(0, 2, 1).reshape(8, 4, 128, 512).transpose(0, 2, 1, 3)
        .reshape(8, 128, 4 * 512)
    )
    # column-type conv1 weights, dw folded: [o, c, dh]
    WL = c1w[:, :, :, 1] + c1w[:, :, :, 2]
    WI = c1w.sum(3)
    WR = c1w[:, :, :, 0] + c1w[:, :, :, 1]
    wc = np.empty((9, 128, 4 * 512), np.float32)
    for t, Wt in enumerate((WL, WI, WR)):
        for dh in range(3):
            wc[t * 3 + dh] = _chunk_pf(np.ascontiguousarray(Wt[:, :, dh].T), 512)
    wc = _bf16(wc)
    # conv2+mean coefficient matrices  C_pq[o, c]
    w = c2w
    C00 = (w[:, :, 0, 0] + w[:, :, 0, 1] + w[:, :, 1, 0] + w[:, :, 1, 1]) / 4
    C01 = (w[:, :, 0, 1] + w[:, :, 0, 2] + w[:, :, 1, 1] + w[:, :, 1, 2]) / 4
    C10 = (w[:, :, 1, 0] + w[:, :, 1, 1] + w[:, :, 2, 0] + w[:, :, 2, 1]) / 4
    C11 = (w[:, :, 1, 1] + w[:, :, 1, 2] + w[:, :, 2, 1] + w[:, :, 2, 2]) / 4
    Cs = (C00, C01, C10, C11)
    c4 = np.empty((4, 128, 4 * 512), np.float32)
    for i, C in enumerate(Cs):
        c4[i] = _chunk_pf(np.ascontiguousarray(C.T), 512)
    c4 = _bf16(c4)

    bias_f = c2b + (C00 + C01 + C10 + C11) @ c1b  # [512]
    with_b1 = bool(np.any(b1))
    with_b2 = bool(np.any(b2))
    with_bf = bool(np.any(bias_f))

    shared = {"w1t": w1t, "w2t": w2t, "wc": wc, "c4": c4}
    if with_b1:
        shared["b1r"] = np.ascontiguousarray(
            np.broadcast_to(b1.reshape(1, 8 * 512), (128, 8 * 512)).astype(np.float32)
        )
    if with_b2:
        shared["b2r"] = np.ascontiguousarray(
            np.broadcast_to(b2.reshape(1, 8 * 512), (128, 8 * 512)).astype(np.float32)
        )
    if with_bf:
        shared["bfr"] = np.ascontiguousarray(
            np.broadcast_to(bias_f, (128, 512)).astype(np.float32)
        )

    in_maps = []
    for c in range(NCORES):
        sl = slice(c * BL, (c + 1) * BL)
        xc = x[sl]  # [128, 1024]
        xt = _bf16(
            xc.T.reshape(8, 128, 128).transpose(1, 0, 2).reshape(128, 8 * 128)
        )
        m = {"xt": xt, "vx": _bf16(vx[sl])}
        m.update(shared)
        in_maps.append(m)
    return in_maps, (with_b1, with_b2, with_bf)


def _run(inputs, trace=False):
    from concourse.bass_utils import run_bass_kernel_spmd

    in_maps, flags = _prep_inputs(**inputs)
    nc = _build_program(*flags)
    res = run_bass_kernel_spmd(
        nc, in_maps, core_ids=list(range(NCORES)), trace=trace
    )
    out = np.empty((B, F), np.float32)
    for c in range(NCORES):
        out[c * BL : (c + 1) * BL] = res.results[c]["out"]
    return out, res


def kernel(**inputs):
    out, _ = _run(inputs, trace=False)
    return out


# revision 44
# speedup vs baseline: 1.0435x; 1.0435x over previous
"""Trainium2 Bass kernel for nn_AttentionModule (B=1024, F=512, N=8).

Pure data-parallel across 8 NeuronCores: batch is sharded 8 x 128, all
weights replicated. No collectives.

Algebraic restructuring (exact, verified vs the jax reference):
  - conv1's input t[b,f,i,j] = va[b,f,i] is constant along j, so the 3x3
    SAME conv over the 8x8 image has only 3 distinct column types
    (q=0 / interior / q=7).  Each type is a 3-tap 1-D conv over the
    height axis with dw-folded weights -> 9 [B*8,512]@[512,512] matmuls.
  - The 4x4/4 maxpool mixes column types pairwise and reduces height
    windows {0..3},{4..7}.
  - conv2 (3x3 SAME on 2x2) + mean + bias collapses to 4 matmuls with
    host-folded coefficient matrices C_pq plus a folded bias vector.
All matmul operands are bf16 (TensorE 1 cyc/row); accumulation f32.
"""

import numpy as np
import ml_dtypes

B, F, N = 1024, 512, 8
NCORES = 8
BL = B // NCORES  # 128 batch rows per core

_BF16 = ml_dtypes.bfloat16


def _bf16(a):
    return np.ascontiguousarray(np.asarray(a, np.float32).astype(_BF16))


def _chunk_pf(mat, free):
    """[K, free] (contraction-major) -> SBUF tile layout [128, (K//128)*free]
    where tile[kp, kc*free + f] = mat[kc*128 + kp, f]."""
    K = mat.shape[0]
    kc = K // 128
    return np.ascontiguousarray(
        mat.reshape(kc, 128, free).transpose(1, 0, 2).reshape(128, kc * free)
    )


def _apply_tile_drain_patch():
    """The neuronxcc walrus in this container rejects instructions with >1
    sync wait (CoreV3 setupSyncWait 'Too many sync wait commands').  Split
    the Tile exit drain's waits across multiple drain instructions."""
    import bass_rust
    import concourse.tile as tile
    from concourse.vector_clock import ScopedClock

    MAXW = 1

    def _drain_and_barrier_split(self, tick_clock, wait_clock):
        nc = self.nc
        drain_inst = nc.sync.drain()
        wait_clock.add_sem_waits(
            drain_inst.ins, ScopedClock({None: tick_clock.global_clock})
        )
        si = drain_inst.ins.sync_info
        waits = list(si.on_wait) if si is not None else []
        if len(waits) > MAXW:
            si.on_wait = waits[:MAXW]
            drain_inst.ins.sync_info = si
            for i in range(MAXW, len(waits), MAXW):
                extra = nc.sync.drain()
                esi = extra.ins.sync_info
                if esi is None:
                    esi = bass_rust.SyncInfo(on_wait=[], on_update=[])
                esi.on_wait = waits[i : i + MAXW]
                extra.ins.sync_info = esi
        nc.all_engine_barrier()
        assert self.sems is not None
        popped = nc._tile_sem_poison_stack.pop()
        assert popped is self._sem_poison
        nc.clear_and_free_semaphores(list(self.sems.allocated().values()))
        nc.all_engine_barrier()

    tile.TileContext._drain_and_barrier = _drain_and_barrier_split


def _split_multi_waits(nc, maxw=1):
    """neuronxcc walrus accepts at most one sync wait per instruction.
    For any instruction carrying more, keep the first wait and hoist the
    rest onto nop instructions inserted immediately before it on the same
    engine."""
    import bass_rust

    for f in nc.m.functions:
        for bb in f.blocks:
            insts = bb.instructions  # live list
            multi = []
            for idx, ins in enumerate(insts):
                si = ins.sync_info
                if si is not None and len(si.on_wait) > maxw:
                    multi.append(idx)
            if not multi:
                continue
            new_list = []
            for idx, ins in enumerate(insts):
                si = ins.sync_info
                waits = list(si.on_wait) if si is not None else []
                if len(waits) > maxw:
                    for j in range(maxw, len(waits), maxw):
                        beng = nc.engines[ins.engine]
                        bi = beng.nop(nofuse=True)
                        cur = nc.cur_bb.bb
                        assert cur.instructions[-1].name == bi.ins.name
                        cur.instructions.pop()
                        nsi = bi.ins.sync_info
                        if nsi is None:
                            nsi = bass_rust.SyncInfo(on_wait=[], on_update=[])
                        nsi.on_wait = waits[j : j + maxw]
                        bi.ins.sync_info = nsi
                        new_list.append(bi.ins)
                    si.on_wait = waits[:maxw]
                    ins.sync_info = si
                new_list.append(ins)
            insts[:] = new_list


def _build_program(with_b1, with_b2, with_bf):
    import concourse.bass as bass
    import concourse.mybir as mybir
    import concourse.tile as tile
    from concourse.masks import make_identity

    _apply_tile_drain_patch()

    bf = mybir.dt.bfloat16
    f32 = mybir.dt.float32
    AF = mybir.ActivationFunctionType
    ALU = mybir.AluOpType

    nc = bass.Bass()
    xt_e = nc.declare_dram_parameter("xt", [128, 8 * 128], bf, isOutput=False)
    vx_e = nc.declare_dram_parameter("vx", [128, 512], bf, isOutput=False)
    w1_e = nc.declare_dram_parameter("w1t", [8, 128, 8 * 512], bf, isOutput=False)
    w2_e = nc.declare_dram_parameter("w2t", [8, 128, 4 * 512], bf, isOutput=False)
    wc_e = nc.declare_dram_parameter("wc", [9, 128, 4 * 512], bf, isOutput=False)
    c4_e = nc.declare_dram_parameter("c4", [4, 128, 4 * 512], bf, isOutput=False)
    if with_b1:
        b1_e = nc.declare_dram_parameter("b1r", [128, 8 * 512], f32, isOutput=False)
    if with_b2:
        b2_e = nc.declare_dram_parameter("b2r", [128, 8 * 512], f32, isOutput=False)
    if with_bf:
        bf_e = nc.declare_dram_parameter("bfr", [128, 512], f32, isOutput=False)
    out_e = nc.declare_dram_parameter("out", [128, 512], f32, isOutput=True)

    with tile.TileContext(nc) as tc:
        with (
            tc.tile_pool(name="const", bufs=1) as constp,
            tc.tile_pool(name="wgt1", bufs=3) as w1p,
            tc.tile_pool(name="wgt2", bufs=3) as w2p,
            tc.tile_pool(name="wc", bufs=9) as wcp,
            tc.tile_pool(name="c4", bufs=4) as c4p,
            tc.tile_pool(name="act", bufs=2) as actp,
            tc.tile_pool(name="gt", bufs=4) as gtp,
            tc.tile_pool(name="pool", bufs=4) as poolp,
            tc.tile_pool(name="psA", bufs=2, space="PSUM") as psA,
            tc.tile_pool(name="psT", bufs=2, space="PSUM") as psT,
            tc.tile_pool(name="psY", bufs=4, space="PSUM") as psY,
        ):
            ident = constp.tile([128, 128], bf, tag="ident")
            make_identity(nc, ident[:])

            xt_sb = constp.tile([128, 8 * 128], bf, tag="xt")
            nc.sync.dma_start(xt_sb[:], xt_e[:])
            vx_sb = constp.tile([128, 512], bf, tag="vx")
            nc.sync.dma_start(vx_sb[:], vx_e[:])
            if with_b1:
                b1_sb = constp.tile([128, 8 * 512], f32, tag="b1")
                nc.sync.dma_start(b1_sb[:], b1_e[:])
            if with_b2:
                b2_sb = constp.tile([128, 8 * 512], f32, tag="b2")
                nc.sync.dma_start(b2_sb[:], b2_e[:])

            wc_sb = [None] * 9
            c4_sb = [None] * 4
            # wc DMA emission staggered between unit-weight DMAs so the
            # first phase-3 blocks (interleaved at p = n-2) have their
            # matrices without starving the unit-weight stream.
            wc_sched = {0: (1, 4, 7), 1: (2, 5, 8), 2: (0, 3, 6)}

            # GT_sb[cc][cp, n*128 + b] = G[b, n, cc*128+cp]  (bf16)
            gt_sb = [
                gtp.tile([128, 8 * 128], bf, tag="gt", name=f"gt{i}")
                for i in range(4)
            ]

            # incremental pooled maxes P00, P01, P10, P11  [128, 512] bf16
            P_sb = [
                poolp.tile([128, 512], bf, tag="P", name=f"P{i}") for i in range(4)
            ]

            def phase3_block(p):
                # y_T[p][b, o] = sum_{dh valid, cc} GT[cc][:, h]^T @ WC[T,dh][cc]
                dhs = [dh for dh in range(3) if 0 <= p - 1 + dh <= 7]
                ys = []
                for t in range(3):  # 0=L, 1=I, 2=R
                    py = psY.tile([128, 512], f32, tag="y", name=f"y{t}_{p}")
                    nmm = len(dhs) * 4
                    i = 0
                    for dh in dhs:
                        h = p - 1 + dh
                        for cc in range(4):
                            nc.tensor.matmul(
                                py[:],
                                gt_sb[cc][:, h * 128 : (h + 1) * 128],
                                wc_sb[t * 3 + dh][:, cc * 512 : (cc + 1) * 512],
                                start=(i == 0),
                                stop=(i == nmm - 1),
                            )
                            i += 1
                    ys.append(py)
                yL, yI, yR = ys
                # yI to SBUF so DVE maxes read one PSUM + one SBUF operand
                yI_sb = actp.tile([128, 512], f32, tag="yis", name=f"yis{p}")
                nc.scalar.activation(yI_sb[:], yI[:], AF.Copy)
                m0 = actp.tile([128, 512], bf, tag="m0", name=f"m0_{p}")
                nc.vector.tensor_tensor(m0[:], yL[:], yI_sb[:], op=ALU.max)
                m1 = actp.tile([128, 512], bf, tag="m1", name=f"m1_{p}")
                nc.vector.tensor_tensor(m1[:], yR[:], yI_sb[:], op=ALU.max)
                w = p // 4
                for idx, m in ((w * 2, m0), (w * 2 + 1, m1)):
                    if p % 4 == 0:
                        nc.vector.tensor_copy(P_sb[idx][:], m[:])
                    else:
                        nc.vector.tensor_tensor(
                            P_sb[idx][:], P_sb[idx][:], m[:], op=ALU.max
                        )

            # ---- phases 1+2 per attention unit n, phase 3 interleaved ----
            for n in range(8):
                w1_sb = w1p.tile([128, 8 * 512], bf, tag="w1")
                nc.sync.dma_start(w1_sb[:, :2048], w1_e[n][:, :2048])
                nc.sync.dma_start(w1_sb[:, 2048:], w1_e[n][:, 2048:])
                w2_sb = w2p.tile([128, 4 * 512], bf, tag="w2")
                nc.sync.dma_start(w2_sb[:], w2_e[n])
                for i in wc_sched.get(n, ()):
                    t = wcp.tile([128, 4 * 512], bf, tag="wc", name=f"wct{i}")
                    nc.sync.dma_start(t[:], wc_e[i])
                    wc_sb[i] = t
                if n == 7:
                    for i in range(4):
                        t = c4p.tile([128, 4 * 512], bf, tag="c4", name=f"c4t{i}")
                        nc.sync.dma_start(t[:], c4_e[i])
                        c4_sb[i] = t
                    if with_bf:
                        bf_sb = constp.tile([128, 512], f32, tag="bf")
                        nc.sync.dma_start(bf_sb[:], bf_e[:])

                # phase 1: h[b, f] = relu(x @ W1[n].T + b1[n])  [128, 512]
                ph = psA.tile([128, 512], f32, tag="acc")
                for kc in range(8):
                    nc.tensor.matmul(
                        ph[:],
                        xt_sb[:, kc * 128 : (kc + 1) * 128],
                        w1_sb[:, kc * 512 : (kc + 1) * 512],
                        start=(kc == 0),
                        stop=(kc == 7),
                    )
                h_sb = actp.tile([128, 512], bf, tag="h")
                if with_b1:
                    htmp = actp.tile([128, 512], f32, tag="htmp")
                    nc.vector.tensor_add(htmp[:], ph[:], b1_sb[:, n * 512 : (n + 1) * 512])
                    nc.scalar.activation(h_sb[:], htmp[:], AF.Relu)
                else:
                    nc.scalar.activation(h_sb[:], ph[:], AF.Relu)

                # transpose h -> hT_sb[fp, fc*128 + b]
                ht_sb = actp.tile([128, 512], bf, tag="ht")
                for fc in range(4):
                    tp = psT.tile([128, 128], bf, tag="tp")
                    nc.tensor.transpose(
                        tp[:], h_sb[:, fc * 128 : (fc + 1) * 128], ident[:]
                    )
                    nc.vector.tensor_copy(ht_sb[:, fc * 128 : (fc + 1) * 128], tp[:])

                # phase 2: att[b, g] = relu(h @ W2[n].T + b2[n])
                pa = psA.tile([128, 512], f32, tag="acc")
                for fc in range(4):
                    nc.tensor.matmul(
                        pa[:],
                        ht_sb[:, fc * 128 : (fc + 1) * 128],
                        w2_sb[:, fc * 512 : (fc + 1) * 512],
                        start=(fc == 0),
                        stop=(fc == 3),
                    )
                att_sb = actp.tile([128, 512], bf, tag="att")
                if with_b2:
                    atmp = actp.tile([128, 512], f32, tag="atmp")
                    nc.vector.tensor_add(atmp[:], pa[:], b2_sb[:, n * 512 : (n + 1) * 512])
                    nc.scalar.activation(att_sb[:], atmp[:], AF.Relu)
                else:
                    nc.scalar.activation(att_sb[:], pa[:], AF.Relu)

                # G[b, c] = att[b, c] * vx[b, c]
                g_sb = actp.tile([128, 512], bf, tag="g")
                nc.vector.tensor_mul(g_sb[:], att_sb[:], vx_sb[:])

                # transpose G into GT_sb[cc][:, n*128:(n+1)*128]
                for cc in range(4):
                    tp = psT.tile([128, 128], bf, tag="tp")
                    nc.tensor.transpose(
                        tp[:], g_sb[:, cc * 128 : (cc + 1) * 128], ident[:]
                    )
                    nc.vector.tensor_copy(
                        gt_sb[cc][:, n * 128 : (n + 1) * 128], tp[:]
                    )

                # phase-3 block p needs GT columns up to h=p+1, i.e. units
                # 0..p+1 done -> run block n-2 here
                if n >= 2:
                    phase3_block(n - 2)

            phase3_block(6)
            phase3_block(7)

            # ---- phase 4: out = relu(sum_pq P_pq @ C_pq^T + bias) ----
            pt_sb = [
                poolp.tile([128, 512], bf, tag="PT", name=f"PT{i}") for i in range(4)
            ]
            for pq in range(4):
                for cc in range(4):
                    tp = psT.tile([128, 128], bf, tag="tp")
                    nc.tensor.transpose(
                        tp[:], P_sb[pq][:, cc * 128 : (cc + 1) * 128], ident[:]
                    )
                    nc.vector.tensor_copy(
                        pt_sb[pq][:, cc * 128 : (cc + 1) * 128], tp[:]
                    )
            po = psA.tile([128, 512], f32, tag="acc")
            i = 0
            for pq in range(4):
                for cc in range(4):
                    nc.tensor.matmul(
                        po[:],
                        pt_sb[pq][:, cc * 128 : (cc + 1) * 128],
                        c4_sb[pq][:, cc * 512 : (cc + 1) * 512],
                        start=(i == 0),
                        stop=(i == 15),
                    )
                    i += 1
            out_sb = constp.tile([128, 512], f32, tag="out")
            if with_bf:
                otmp = constp.tile([128, 512], f32, tag="otmp")
                nc.vector.tensor_add(otmp[:], po[:], bf_sb[:])
                nc.scalar.activation(out_sb[:], otmp[:], AF.Relu)
            else:
                nc.scalar.activation(out_sb[:], po[:], AF.Relu)
            nc.sync.dma_start(out_e[:], out_sb[:])

    _split_multi_waits(nc)
    return nc


def _prep_inputs(vx, ax, W1, b1, W2, b2, conv1_w, conv1_b, conv2_w, conv2_b):
    vx = np.asarray(vx, np.float32)
    ax = np.asarray(ax, np.float32)
    W1 = np.asarray(W1, np.float32)
    b1 = np.asarray(b1, np.float32)
    W2 = np.asarray(W2, np.float32)
    b2 = np.asarray(b2, np.float32)
    c1w = np.asarray(conv1_w, np.float32)
    c1b = np.asarray(conv1_b, np.float32)
    c2w = np.asarray(conv2_w, np.float32)
    c2b = np.asarray(conv2_b, np.float32)

    x = np.concatenate([vx, ax], axis=1)  # [B, 2F]

    # shared weights
    w1t = _bf16(
        W1.transpose(0, 2, 1).reshape(8, 8, 128, 512).transpose(0, 2, 1, 3)
        .reshape(8, 128, 8 * 512)
    )
    w2t = _bf16(
        W2.transpose(0, 2, 1).reshape(8, 4, 128, 512).transpose(0, 2, 1, 3)
        .reshape(8, 128, 4 * 512)
    )
    # column-type conv1 weights, dw folded: [o, c, dh]
    WL = c1w[:, :, :, 1] + c1w[:, :, :, 2]
    WI = c1w.sum(3)
    WR = c1w[:, :, :, 0] + c1w[:, :, :, 1]
    wc = np.empty((9, 128, 4 * 512), np.float32)
    for t, Wt in enumerate((WL, WI, WR)):
        for dh in range(3):
            wc[t * 3 + dh] = _chunk_pf(np.ascontiguousarray(Wt[:, :, dh].T), 512)
    wc = _bf16(wc)
    # conv2+mean coefficient matrices  C_pq[o, c]
    w = c2w
    C00 = (w[:, :, 0, 0] + w[:, :, 0, 1] + w[:, :, 1, 0] + w[:, :, 1, 1]) / 4
    C01 = (w[:, :, 0, 1] + w[:, :, 0, 2] + w[:, :, 1, 1] + w[:, :, 1, 2]) / 4
    C10 = (w[:, :, 1, 0] + w[:, :, 1, 1] + w[:, :, 2, 0] + w[:, :, 2, 1]) / 4
    C11 = (w[:, :, 1, 1] + w[:, :, 1, 2] + w[:, :, 2, 1] + w[:, :, 2, 2]) / 4
    Cs = (C00, C01, C10, C11)
    c4 = np.empty((4, 128, 4 * 512), np.float32)
    for i, C in enumerate(Cs):
        c4[i] = _chunk_pf(np.ascontiguousarray(C.T), 512)
    c4 = _bf16(c4)

    bias_f = c2b + (C00 + C01 + C10 + C11) @ c1b  # [512]
    with_b1 = bool(np.any(b1))
    with_b2 = bool(np.any(b2))
    with_bf = bool(np.any(bias_f))

    shared = {"w1t": w1t, "w2t": w2t, "wc": wc, "c4": c4}
    if with_b1:
        shared["b1r"] = np.ascontiguousarray(
            np.broadcast_to(b1.reshape(1, 8 * 512), (128, 8 * 512)).astype(np.float32)
        )
    if with_b2:
        shared["b2r"] = np.ascontiguousarray(
            np.broadcast_to(b2.reshape(1, 8 * 512), (128, 8 * 512)).astype(np.float32)
        )
    if with_bf:
        shared["bfr"] = np.ascontiguousarray(
            np.broadcast_to(bias_f, (128, 512)).astype(np.float32)
        )

    in_maps = []
    for c in range(NCORES):
        sl = slice(c * BL, (c + 1) * BL)
        xc = x[sl]  # [128, 1024]
        xt = _bf16(
            xc.T.reshape(8, 128, 128).transpose(1, 0, 2).reshape(128, 8 * 128)
        )
        m = {"xt": xt, "vx": _bf16(vx[sl])}
        m.update(shared)
        in_maps.append(m)
    return in_maps, (with_b1, with_b2, with_bf)


def _run(inputs, trace=False):
    from concourse.bass_utils import run_bass_kernel_spmd

    in_maps, flags = _prep_inputs(**inputs)
    nc = _build_program(*flags)
    res = run_bass_kernel_spmd(
        nc, in_maps, core_ids=list(range(NCORES)), trace=trace
    )
    out = np.empty((B, F), np.float32)
    for c in range(NCORES):
        out[c * BL : (c + 1) * BL] = res.results[c]["out"]
    return out, res


def kernel(**inputs):
    out, _ = _run(inputs, trace=False)
    return out
